# revision 139
# baseline (speedup 1.0000x reference)
"""Single-head causal attention on 8 trn2 NeuronCores (Bass/Tile).

Problem: x [4, 4096, 768] f32; Wk/Wq/Wv [768, 64]; out = softmax(causal(q k^T/8)) v.

Sharding: 8 cores = 4 batches x 2 cores. Per batch the 32 query tiles
(128 rows each) are split between its 2 cores so that BOTH cores run the
IDENTICAL program: slot s (s=0..15) processes one q-tile against a key
prefix of exactly L[s] = 256*(s+1) local keys.  Core h=0 takes global
q-tiles {0,3,4,7,8,...}; core h=1 takes {1,2,5,6,9,...} and gets its x
PERMUTED on host (128-row tile pairs 2a<->2a+1 swapped) so the slot->tile
mapping is position-identical across cores.  Only the last 256 keys of
each slot need a mask, which the host precomputes.

Per-core compute (per 512-key piece p = slot pair 2p/2p+1):
  - k/v co-projected on PE in fp16 (fp32 PSUM); q projected for the 2
    local tiles.  One DVE copy evacuates [k;v] to fp16 SBUF.
  - k is split hi+lo fp8e4 on the otherwise-idle GPSIMD (SBUF->SBUF).
    Scores are DoubleRow fp8 matmuls - (k_hi, k_lo) as the two dual-row
    tiles, fp8 q duplicated via a stride-0 AP - which the cost model
    prices at 0.5 cycles/column: 2x fewer PE cycles than f32r, with
    fp16-level k precision (only q carries fp8 quantization error).
  - exp runs on ACT (the critical engine: ~37us busy = the wall) with
    bias -3.0 (softmax shift-invariant; keeps exp <= e^4.5 << 240 =
    fp8e4 max) writing fp8 weights directly.
  - causal masks are multiplicative {1,0} fp8, applied POST-exp to the
    weights on DVE so they never sit on the ACT critical path; the mask
    emission is deferred to just before the masked group's AV.
  - v^T via 4 fp16 PE transposes (fp16 PSUM), split hi+lo fp8 on DVE;
    AV accumulates outT [65, 256] with DoubleRow (v_hi, v_lo) dual-row
    tiles and stride-0-duplicated weights; the ones-column of v_hi
    (zeros in v_lo) yields softmax denominators for free.
  - piece 0 (shortest prefixes, error-amplification-critical) uses an
    exact f32r score/exp/AV path instead.

Schedule: a flat cross-piece software pipeline - each exp slot emits one
score group, the AV of the previous slot's group, and drains a couple of
small projection/transpose emission bundles (work queue) so no multi-us
PE blob ever sits between score groups and the ACT exp stream stays fed.
The final piece runs its masked group FIRST to shorten the drain tail.
x/mask DMAs ride a prefetch-only SP queue (stores go via Pool SWDGE, the
final one via SP); ~14 dummy DoubleRow matmuls on memset data warm the
PE p-state ramp through the initial DMA window.

Host divides by the denominator row and unpermutes.  Measured on HW:
rel err 1.007e-2 (tolerance 2e-2); TimelineSim 54.6us vs 76us baseline.
"""

import functools
import sys

import ml_dtypes
import numpy as np

if "/opt/trn_rl_repo" not in sys.path:
    sys.path.insert(0, "/opt/trn_rl_repo")

B, T, C, H = 4, 4096, 768, 64
NCHUNK = C // 128          # 6 embedding chunks
NSLOT = 16                 # q-tiles per core
NPAIR = 8                  # slot pairs / xT pieces
NEG = -120.0               # fp8e4-exact; exp(NEG+s-2.5) == 0 in fp32
EBIAS = -3.0               # exp bias: max score ~7.4 -> wt < 240 (fp8e4 max)

# local q-tile index per slot: even s -> 2s, odd s -> 2s+1
LTS = [2 * s + (s % 2) for s in range(NSLOT)]


def _build_program():
    import concourse.bass as bass
    import concourse.tile as tile
    from concourse import mybir
    from contextlib import ExitStack

    f32r = mybir.dt.float32r
    f32 = mybir.dt.float32
    f16 = mybir.dt.float16
    fp8 = mybir.dt.float8e4
    EXP = mybir.ActivationFunctionType.Exp
    DR = mybir.MatmulPerfMode.DoubleRow

    nc = bass.Bass(trn_type="TRN2", target_bir_lowering=False, debug=False)

    def dup2(ap, n):
        """[P, n] AP read twice -> [P, 2, n] via a stride-0 middle dim."""
        return bass.AP(tensor=ap.tensor, offset=ap.offset,
                       ap=[ap.ap[0], [0, 2], [1, n]])

    # host-prepared layouts (p = partition index)
    xT = nc.dram_tensor("xT", [128, NCHUNK, T], f16, kind="ExternalInput").ap()
    wkv = nc.dram_tensor("wkv", [128, NCHUNK, 128], f16, kind="ExternalInput").ap()
    wq = nc.dram_tensor("wq", [128, NCHUNK, H], f16, kind="ExternalInput").ap()
    maskT = nc.dram_tensor(
        "maskT", [128, NPAIR, 6, 128], fp8, kind="ExternalInput"
    ).ap()
    ident_d = nc.dram_tensor("ident", [128, 64], f16, kind="ExternalInput").ap()
    # output stays transposed [h, slot*128+q] with the softmax-denominator
    # row appended (row 64); host divides + untransposes
    out_d = nc.dram_tensor(
        "out", [H + 1, NSLOT * 128], f16, kind="ExternalOutput"
    ).ap()

    with ExitStack() as ctx:
        tc = ctx.enter_context(tile.TileContext(nc))
        const = ctx.enter_context(tc.tile_pool(name="const", bufs=1))
        xp_pool = ctx.enter_context(tc.tile_pool(name="xp", bufs=3))
        kv_pool = ctx.enter_context(tc.tile_pool(name="kv16", bufs=2))
        q8_pool = ctx.enter_context(tc.tile_pool(name="q8", bufs=2))
        wt_pool = ctx.enter_context(tc.tile_pool(name="wt", bufs=9))
        ot_pool = ctx.enter_context(tc.tile_pool(name="oT", bufs=3))
        ps_kv = ctx.enter_context(tc.tile_pool(name="pskv", bufs=1, space="PSUM"))
        ps_vt = ctx.enter_context(tc.tile_pool(name="psvt", bufs=1, space="PSUM"))
        ps_sc = ctx.enter_context(tc.tile_pool(name="pssc", bufs=2, space="PSUM"))
        ps_q = ctx.enter_context(tc.tile_pool(name="psq", bufs=1, space="PSUM"))
        ps_o = ctx.enter_context(tc.tile_pool(name="pso", bufs=1, space="PSUM"))

        wkv_s = const.tile([128, NCHUNK, 128], f16)
        wq_s = const.tile([128, NCHUNK, H], f16)
        # mask slices are DMA'd per-piece so the transfer doesn't delay the
        # first x pieces at startup
        mask_s = const.tile([128, NPAIR, 6, 128], fp8)
        # identity stacked twice so ident[64:128] sits at base partition 64
        # (transpose requires stationary/moving at the same base partition)
        ident = const.tile([128, 64], f16)

        # persistent attention state
        k8 = const.tile([64, 2, T], fp8)          # keys^T fp8 (hi, lo)
        vaug8 = const.tile([128, 2, T // 128, 80], fp8)  # vT (hi, lo) + ones
        u8 = mybir.dt.uint8
        warm8 = const.tile([128, 384], fp8)
        nc.gpsimd.memset(warm8.bitcast(u8), 0x38)
        nc.gpsimd.memset(vaug8[:, 0, :, H : H + 1].bitcast(u8), 0x38)  # fp8 1.0
        nc.gpsimd.memset(vaug8[:, 1, :, H : H + 1].bitcast(u8), 0)
        # piece-0 exact-path (f32r) state
        kR = const.tile([64, 512], f32r)
        qR = const.tile([64, 256], f32r)
        vaugR = const.tile([128, 4, 80], f32r)
        nc.gpsimd.memset(vaugR[:, :, H : H + 1].bitcast(f32), 1.0)

        bias_t = const.tile([128, 1], f32)
        nc.gpsimd.memset(bias_t, EBIAS)
        # preload the exp table set during the startup DMA window
        warm = const.tile([128, 1], f32r)
        nc.scalar.activation(warm, bias_t, EXP)

        def emit_xp(p, nsplit=2):
            # ---- load xT piece p: [128, 6, 512] (keys 512p..512p+512) ----
            xp = xp_pool.tile([128, NCHUNK, 512], f16, tag="xp")
            step = NCHUNK // nsplit
            for i in range(0, NCHUNK, step):
                nc.sync.dma_start(
                    out=xp[:, i : i + step, :],
                    in_=xT[:, i : i + step, p * 512 : (p + 1) * 512])
            # mask slice for piece p-1 rides behind xp(p): small enough to
            # never delay the x prefetch stream materially, early enough to
            # land before piece p-1's masked AV
            if p >= 1:
                nc.sync.dma_start(out=mask_s[:, p - 1, :, :],
                                  in_=maskT[:, p - 1, :, :])
            if p == NPAIR - 1:
                nc.sync.dma_start(out=mask_s[:, p, :, :],
                                  in_=maskT[:, p, :, :])
            return xp

        proj = {}        # p -> (q8, kv16), filled by the evac thunk

        def emit_kv_chunks(p, xp, kv_ps, cs):
            # kv projection: [Wk|Wv]^T @ x^T -> PSUM [128, 512]
            for c in cs:
                nc.tensor.matmul(
                    kv_ps,
                    lhsT=wkv_s[:, c, :],
                    rhs=xp[:, c, :],
                    start=(c == 0),
                    stop=(c == NCHUNK - 1),
                    skip_group_check=True,
                )

        def emit_q_chunks(p, xp, q_ps, cs):
            # q projection for local tiles 4p, 4p+3 (slots 2p, 2p+1)
            for c in cs:
                base = xp[:, c, 0:128]
                q_rhs = bass.AP(
                    tensor=base.tensor,
                    offset=base.offset,
                    ap=[base.ap[0], [384, 2], [1, 128]],
                )
                nc.tensor.matmul(
                    q_ps,
                    lhsT=wq_s[:, c, :],
                    rhs=q_rhs,
                    start=(c == 0),
                    stop=(c == NCHUNK - 1),
                    skip_group_check=True,
                )

        def emit_evac(p, kv_ps, q_ps):
            # evacuate projections + k hi/lo fp8 on GPSIMD (SBUF->SBUF).
            # q8 first: it gates the next piece's first score group, while
            # kv16 only feeds the (later) k-quant / transposes
            q8 = q8_pool.tile([64, 256], fp8, tag="q8")
            nc.vector.tensor_copy(q8, q_ps)
            if p == 0:
                nc.vector.tensor_copy(kR, kv_ps[0:64, :])
                nc.vector.tensor_copy(qR, q_ps)
            kv16 = kv_pool.tile([128, 512], f16, tag="kv16")
            nc.vector.tensor_copy(kv16, kv_ps)
            kcols = slice(p * 512, (p + 1) * 512)
            nc.gpsimd.tensor_copy(k8[:, 0, kcols], kv16[0:64, :])
            nc.gpsimd.tensor_sub(k8[:, 1, kcols], kv16[0:64, :], k8[:, 0, kcols])
            proj[p] = (q8, kv16)

        def proj_thunks(p, xp):
            """Small emission bundles for piece p's projections, drained
            a little per exp slot so no multi-us PE blob ever sits between
            two score-group emissions."""
            kv_ps = ps_kv.tile([128, 512], f32, tag="kv")
            q_ps = ps_q.tile([64, 256], f32, tag="q")
            return [
                lambda: emit_kv_chunks(p, xp, kv_ps, range(0, 2)),
                lambda: emit_kv_chunks(p, xp, kv_ps, range(2, 4)),
                lambda: emit_kv_chunks(p, xp, kv_ps, range(4, 6)),
                lambda: emit_q_chunks(p, xp, q_ps, range(0, 3)),
                lambda: emit_q_chunks(p, xp, q_ps, range(3, 6)),
                lambda: emit_evac(p, kv_ps, q_ps),
            ]

        def vtrans_thunks(p):
            """v^T via 4 fp16 PE transposes into PSUM fp16, then v hi/lo
            fp8 on DVE.  kv16(p) was evacuated during piece p-1."""
            vt_ps = ps_vt.tile([128, 4, 64], f16, tag="vt")

            def tr(js):
                kv16 = proj[p][1]
                for j in js:
                    nc.tensor.transpose(vt_ps[:, j, :],
                                        kv16[64:128, j * 128 : (j + 1) * 128],
                                        ident[64:128, :])

            def vquant():
                vch = slice(4 * p, 4 * p + 4)
                nc.vector.tensor_copy(vaug8[:, 0, vch, 0:H], vt_ps)
                nc.vector.tensor_sub(vaug8[:, 1, vch, 0:H], vt_ps,
                                     vaug8[:, 0, vch, 0:H])
                if p == 0:
                    nc.vector.tensor_copy(vaugR[:, :, 0:H], vt_ps)

            return [lambda: tr((0, 1)), lambda: tr((2, 3)), vquant]

        def emit_scores_exp(p, g, q8):
            exact = p == 0
            scg = ps_sc.tile([128, 4, 256], f32, tag="sc")
            for j in range(4):
                kc = 4 * g + j
                if exact:
                    nc.tensor.matmul(
                        scg[:, j, :],
                        lhsT=kR[:, kc * 128 : (kc + 1) * 128],
                        rhs=qR,
                        start=True, stop=True,
                        skip_group_check=True,
                    )
                else:
                    nc.tensor.matmul(
                        scg[:, j, :],
                        lhsT=k8[:, :, kc * 128 : (kc + 1) * 128],
                        rhs=dup2(q8, 256),
                        start=True, stop=True,
                        perf_mode=DR,
                        skip_group_check=True,
                    )
            wt = wt_pool.tile([128, 4, 256], f32r if exact else fp8,
                              tag="wtR" if exact else "wt")
            nc.scalar.activation(wt, scg, EXP, bias=bias_t)
            if g != p:
                return wt, None

            def mask_mul():
                # post-exp multiplicative {1, 0} window masks on DVE: off
                # the ACT critical path.  exp of unmasked garbage is safe:
                # scores <= ~7.5 -> wt <= e^4.5 = 90 < 240 (fp8e4 max).
                # Emitted just before this group's AV so the DVE queue isn't
                # blocked ahead of the next piece's projection evacuations.
                nc.vector.tensor_mul(
                    wt[:, 0:4, 0:128], wt[:, 0:4, 0:128],
                    mask_s[:, p, 0:4, :],
                )
                nc.vector.tensor_mul(
                    wt[:, 2:4, 128:256], wt[:, 2:4, 128:256],
                    mask_s[:, p, 4:6, :],
                )
            return wt, mask_mul

        def emit_av(p, g, wt, outT_ps, first, last):
            exact = p == 0
            for j in range(4):
                kc = 4 * g + j
                if exact:
                    nc.tensor.matmul(
                        outT_ps,
                        lhsT=vaugR[:, kc, 0:65],
                        rhs=wt[:, j, :],
                        start=(first and j == 0),
                        stop=(last and j == 3),
                        skip_group_check=True,
                    )
                else:
                    nc.tensor.matmul(
                        outT_ps,
                        lhsT=vaug8[:, :, kc, 0:65],
                        rhs=dup2(wt[:, j, :], 256),
                        start=(first and j == 0),
                        stop=(last and j == 3),
                        perf_mode=DR,
                        skip_group_check=True,
                    )

        def emit_store(p, outT_ps, final=False):
            outT_s = ot_pool.tile([H + 1, 256], f16, tag="oTs")
            nc.vector.tensor_copy(outT_s, outT_ps)
            # SWDGE (Pool) keeps stores off the prefetch queue; the final
            # store uses the now-idle SP HWDGE (faster desc-gen)
            eng = nc.sync if final else nc.gpsimd
            eng.dma_start(out=out_d[:, p * 256 : (p + 1) * 256], in_=outT_s)

        # Bootstrap: the tiny wkv transfer goes first (it gates the first
        # kv chunk), then the xp(0) slices; piece 0's projections run
        # un-spread.  Meanwhile ~20 dummy DoubleRow matmuls on memset data
        # keep PE busy through the DMA window so the p-state ramp reaches
        # full clock before the first real projection.
        nc.sync.dma_start(out=wkv_s, in_=wkv)
        xp0 = emit_xp(0, nsplit=3)
        nc.sync.dma_start(out=wq_s, in_=wq)
        xp1 = emit_xp(1)
        nc.sync.dma_start(out=ident, in_=ident_d)
        scw = ps_sc.tile([128, 4, 256], f32, tag="sc")

        def warm_mm(n):
            for _ in range(n):
                nc.tensor.matmul(
                    scw[:, 0, :],
                    lhsT=bass.AP(tensor=warm8.tensor, offset=warm8.offset,
                                 ap=[warm8.ap[0], [128, 2], [1, 128]]),
                    rhs=bass.AP(tensor=warm8.tensor, offset=warm8.offset,
                                ap=[warm8.ap[0], [0, 2], [1, 256]]),
                    start=True, stop=True,
                    perf_mode=DR,
                    skip_group_check=True,
                )

        warm_mm(14)
        for t in proj_thunks(0, xp0):
            t()

        # Flat cross-piece software pipeline: each exp slot emits one score
        # group + the AV of the previous slot's group, then drains a couple
        # of queued projection/transpose bundles, so ACT's exp stream stays
        # fed and PE work is spread evenly.
        work = []
        pend_q = []                  # groups awaiting AV (depth-2 flush)
        outT = {}

        def flush_one():
            pp, gg, wtt, mm, ff, ll = pend_q.pop(0)
            if mm is not None:
                mm()
            emit_av(pp, gg, wtt, outT[pp], ff, ll)
            if ll:
                emit_store(pp, outT[pp], final=len(pend_q) == 0 and
                           pp == NPAIR - 1)

        for p in range(NPAIR):
            q8_p = proj[p][0]
            if p + 1 < NPAIR:
                work.extend(proj_thunks(p + 1, xp1 if p == 0 else
                                        emit_xp(p + 1)))
            work.extend(vtrans_thunks(p))
            if p == 0:
                # start piece 1's kv projection ahead of piece 0's scores:
                # it only needs xp(1), which lands before kR/qR are ready
                for _ in range(3):
                    work.pop(0)()

            # masked group first on the final piece: shortens the
            # post-last-exp tail (mask muls + AV hide under earlier slots)
            order = ([p] + list(range(p))) if p == NPAIR - 1 else \
                list(range(p + 1))
            avflags = {g: (i == 0, i == p) for i, g in enumerate(order)}
            for i, g in enumerate(order):
                wt, mul = emit_scores_exp(p, g, q8_p)
                if i == 0:
                    oT_tile = ps_o.tile([H + 1, 256], f32, tag="oT")
                    outT[p] = oT_tile
                depth = 7
                while len(pend_q) >= depth:
                    flush_one()
                # last piece: masked group runs FIRST, so its v hi/lo
                # quant must be fully drained before its AV is flushed
                ndrain = len(work) if p == NPAIR - 1 else \
                    -(-len(work) // (p + 1 - i))  # ceil
                for _ in range(ndrain):
                    work.pop(0)()
                pend_q.append((p, g, wt, mul) + avflags[g])
        while pend_q:
            flush_one()

    _split_matmul_waits(nc, mybir)
    return nc


def _split_matmul_waits(nc, mybir):
    """Several TRN2 instruction structs carry only ONE sync-wait slot
    (walrus: "Too many sync wait commands").  Hoist extra waits onto a
    chain of InstNoOps inserted immediately before, on the same engine —
    in-order execution preserves the semantics."""
    k = 0
    skip = {"InstAllEngineBarrier", "InstNoOp"}
    for f in nc.m.functions:
        for blk in f.blocks:
            il = blk.instructions
            i = 0
            while i < len(il):
                inst = il[i]
                if type(inst).__name__ not in skip:
                    si = inst.sync_info
                    waits = list(si.on_wait) if si is not None and si.on_wait else []
                    if len(waits) > 1:
                        for w in waits[:-1]:
                            nop = mybir.InstNoOp(
                                name=f"I-waitfix-{k}",
                                engine=inst.engine,
                                sync_info=mybir.SyncInfo(
                                    on_wait=[w], on_update=[]
                                ),
                            )
                            k += 1
                            il.insert(i, nop)
                            i += 1
                        inst.sync_info = mybir.SyncInfo(
                            on_wait=waits[-1:], on_update=list(si.on_update or [])
                        )
                i += 1


@functools.lru_cache(maxsize=1)
def _get_program():
    return _build_program()


def _diag_block(lo, hi):
    j = np.arange(128)[:, None]
    i = np.arange(128)[None, :]
    return np.where(j <= i, lo, hi).astype(np.float32)


def _host_inputs(x, Wk, Wq, Wv):
    """Build per-core input dicts."""
    e4 = ml_dtypes.float8_e4m3

    wkv_h = (
        np.concatenate([Wk, Wv], axis=1)
        .reshape(NCHUNK, 128, 128)
        .transpose(1, 0, 2)
        .astype(np.float16)
    )
    wq_h = (
        (Wq / np.sqrt(H)).reshape(NCHUNK, 128, H).transpose(1, 0, 2)
        .astype(np.float16)
    )
    in_maps = []
    for core in range(8):
        b, h = core // 2, core % 2
        xt = np.asarray(x[b]).reshape(T // 128, 128, C)
        if h == 1:
            perm = np.arange(T // 128) ^ 1
            xt = xt[perm]
        # [p, c, t] = xperm[t, c*128+p]
        xT_h = (
            xt.reshape(T, C).T.reshape(NCHUNK, 128, T).transpose(1, 0, 2)
            .astype(np.float16)
        )
        # masks [128, NPAIR, 6, 128]: quarters 0-3 = slot 2p over the last
        # group's 4 key-tiles (left halves), quarters 4-5 = slot 2p+1 over
        # its 2 window tiles (right halves).  Multiplicative {1, 0},
        # applied post-exp to the weights.
        mask_h = np.empty((128, NPAIR, 6, 128), np.float32)
        for p in range(NPAIR):
            keep, kill = 1.0, 0.0
            diag = _diag_block(keep, kill)

            def blk(s, t):
                qg = LTS[s] ^ h
                kg = t ^ h
                if kg == qg:
                    return diag
                return np.full((128, 128), keep if kg < qg else kill,
                               np.float32)

            for j in range(4):
                mask_h[:, p, j, :] = blk(2 * p, 4 * p + j)
            for j in range(2):
                mask_h[:, p, 4 + j, :] = blk(2 * p + 1, 4 * p + 2 + j)
        in_maps.append(
            {"xT": xT_h, "wkv": wkv_h, "wq": wq_h,
             "maskT": mask_h.astype(e4),
             "ident": np.concatenate([np.eye(64), np.eye(64)]).astype(np.float16)}
        )
    return in_maps


def _unshard(results):
    out = np.empty((B, T, H), np.float32)
    for core in range(8):
        b, h = core // 2, core % 2
        oc = results[core]["out"].astype(np.float32)  # [H+1, NSLOT*128]
        oc = (oc[:H] / oc[H : H + 1]).reshape(H, NSLOT, 128)
        ob = out[b].reshape(T // 128, 128, H)
        for s in range(NSLOT):
            ob[LTS[s] ^ h] = oc[:, s, :].T
    return out


def kernel(x, Wk, Wq, Wv):
    from concourse import bass_utils

    nc = _get_program()
    in_maps = _host_inputs(
        np.asarray(x, np.float32),
        np.asarray(Wk, np.float32),
        np.asarray(Wq, np.float32),
        np.asarray(Wv, np.float32),
    )
    res = bass_utils.run_bass_kernel_spmd(nc, in_maps, core_ids=list(range(8)))
    return _unshard(res.results)
